# revision 11
# baseline (speedup 1.0000x reference)
"""Trainium2 Bass kernel for ContextQuestionAttention (BiDAF-style).

Reference computation (per example):
    w1, w2, w3 = w[:H], w[H:2H], w[2H:]
    S[i,j] = C[i]·w1 + Q[j]·w2 + sum_h C[i,h] Q[j,h] w3[h]
    S = where(q_mask==0, -1e9, S)
    A = softmax_j(S) @ Q
    B_att = softmax_i(max_j S); B_vec = B_att @ C
    out = concat([C, A, C*A, C*B_vec], -1)

Sharding: data-parallel over batch, 4 examples per core on 8 cores.

Kernel strategy (per example):
  - All big matmuls/transposes run as float32r (TF32-class PE fast path,
    1 cycle/row vs 4 for fp32) via AP bitcast; PSUM accumulation is fp32.
  - The out[:, 0:H] = C chunk is written by one 2MB DMA straight from the
    C tiles in SBUF, right after the load - zero compute dependency, so it
    streams while the current example's attention math runs.
  - V^T[j,i] = s_cq^T + s_c[i] + s_q[j] + maskbias[j], with j on partitions:
    4 accumulating matmuls (lhsT=(w3*Q)^T chunks, rhs=C^T chunks) plus one
    K=1 rank-1 matmul (lhsT=ones_row, rhs=s_c row) folding the s_c term; the
    per-partition (s_q + mask) bias is folded into the Exp activation that
    yields P'T = exp(V^T).  Since s_c is inside, rowmax of P' IS
    E = exp(maxS) directly (softmax_j is unaffected - e^{s_c} cancels).
  - s_c row [1, 1024] via M=1 matmuls: lhsT = w1 chunk [128,1], rhs = C^T.
  - A[i,:] = P'T[:,i].T @ Q / Z'[i]; Z' via ones-column matmuls.
  - B_vec row [1, H] via M=1 matmuls contracting i: lhsT = E column,
    rhs = C tiles; scaled by 1/sum(E) and broadcast across partitions with
    a K=1 matmul.
  - exp() without max subtraction is safe: |S| <~ 15 for these magnitudes.
  - Each column half is processed end-to-end (C^T -> S^T -> exp -> rowmax ->
    Z' -> A), so [A|C*A] i-tiles start streaming out (4KB rows) about half a
    prep earlier than a monolithic pass would allow; C*B tiles follow once
    the B path completes.  Input loads issue on the ACT HWDGE queue and
    stores on the SP queue so a store's unsatisfied semaphore never
    head-of-line blocks a load.
"""

import os
import sys
from contextlib import ExitStack

import numpy as np

for _p in ("/opt/trn_rl_repo", "/root/.axon_site/_ro/trn_rl_repo"):
    if os.path.isdir(_p) and _p not in sys.path:
        sys.path.append(_p)

import concourse.bass as bass
import concourse.tile as tile
from concourse import bacc, mybir
from concourse.bass_utils import run_bass_kernel_spmd

F32 = mybir.dt.float32
F32R = mybir.dt.float32r
I32 = mybir.dt.int32
AX = mybir.AxisListType
ALU = mybir.AluOpType
ACTF = mybir.ActivationFunctionType
ts = bass.ts

N_CORES = 8
B_TOTAL = 32
B_PER_CORE = B_TOTAL // N_CORES  # 4
CLEN = 1024
QLEN = 128
H = 512
NT = CLEN // 128  # 8 i-tiles per example
KH = H // 128     # 4 h-chunks


def _r(ap):
    """fp32 -> float32r reinterpret (PE fast path)."""
    return ap.bitcast(F32R)


def _emit_example(nc, pools, aps, b):
    (c_pool, ct_pool, q_pool, pt_pool, sm_pool, scr_pool, a_pool, ot_pool,
     p_mm, p_ct, p_sm, p_ty) = pools
    C_ap, Q_ap, M_ap, O_ap, consts = aps
    ident, ones_row, ones_col, w1c, w3c, W2b = consts

    # ---- loads: C in four 1MB DMAs; each covers two i-tiles ----
    call = c_pool.tile([128, NT * H], F32, tag="call", bufs=3)
    cq = C_ap[b].rearrange("(g t p) h -> g p t h", g=4, p=128)
    for g in range(4):
        nc.scalar.dma_start(_r(call[:, ts(g, NT * H // 4)]), cq[g].bitcast(F32R))
    Csb = [call[:, ts(t, H)] for t in range(NT)]
    Qsb = q_pool.tile([128, H], F32, tag="q", bufs=2)
    nc.scalar.dma_start(_r(Qsb[:]), Q_ap[b].bitcast(F32R))
    msk = sm_pool.tile([128, 1], I32, tag="msk", bufs=2)
    nc.scalar.dma_start(msk[:], M_ap[b].rearrange("(p a) -> p a", a=1))

    # ---- out[:, 0:H] = C: one 2MB DMA, no compute dependency ----
    nc.sync.dma_start(
        O_ap[b][:, 0:H].rearrange("(t p) h -> p t h", p=128), call[:])

    # ---- mask bias + s_q (per-partition over j) ----
    mskf = sm_pool.tile([128, 1], F32, tag="mskf", bufs=2)
    nc.vector.tensor_copy(mskf[:], msk[:])
    mb = sm_pool.tile([128, 1], F32, tag="mb", bufs=2)
    # (mask - 1) * 1e9  -> 0 where mask==1, -1e9 where mask==0
    nc.vector.tensor_scalar(
        out=mb[:], in0=mskf[:], scalar1=1.0, scalar2=1.0e9,
        op0=ALU.subtract, op1=ALU.mult)
    scr = scr_pool.tile([128, H], F32, tag="scr", bufs=2)
    sq = sm_pool.tile([128, 1], F32, tag="sq", bufs=2)
    sqe = sm_pool.tile([128, 1], F32, tag="sqe", bufs=2)
    # sqe[j] = mb[j] + sum_h Q[j,h] * w2[h]
    nc.vector.tensor_mul(scr[:], Qsb[:], W2b[:])
    nc.vector.reduce_sum(sq[:], scr[:], axis=AX.X)
    nc.vector.tensor_add(sqe[:], sq[:], mb[:])

    # ---- (w3 * Q)^T chunks ----
    QW3T = q_pool.tile([128, H], F32, tag="qw3t", bufs=2)
    for k in range(KH):
        pqt = p_sm.tile([128, 128], F32, tag="sm", bufs=2)
        nc.tensor.transpose(_r(pqt[:]), _r(Qsb[:, ts(k, 128)]), _r(ident[:]))
        nc.vector.tensor_scalar_mul(_r(QW3T[:, ts(k, 128)]), pqt[:], w3c[k][:])

    CT = [ct_pool.tile([128, CLEN], F32, tag=f"ct{k}", bufs=2,
                       name=f"ct{k}_{b}")
          for k in range(KH)]
    scrow = sm_pool.tile([1, CLEN], F32, tag="scrow", bufs=2)
    PT = pt_pool.tile([128, CLEN], F32, tag="pt", bufs=2)
    E = sm_pool.tile([128, NT], F32, tag="e", bufs=2)
    RZP = sm_pool.tile([128, NT], F32, tag="rzp", bufs=2)

    # ---- C^T chunks (PE transposes, batched 4-per-psum-bank) ----
    for half in range(2):
        for k in range(KH):
            pct = p_ct.tile([128, 512], F32, tag="ct", bufs=2)
            for tt in range(4):
                t = half * 4 + tt
                nc.tensor.transpose(
                    _r(pct[:, ts(tt, 128)]), _r(Csb[t][:, ts(k, 128)]),
                    _r(ident[:]))
            if k % 2 == 0:
                nc.scalar.copy(_r(CT[k][:, ts(half, 512)]), pct[:])
            else:
                nc.vector.tensor_copy(_r(CT[k][:, ts(half, 512)]), pct[:])

    # ---- per column half: s_c, S^T+exp, rowmax, Z', A, [A|C*A] out ----
    for half in range(2):
        # s_c row on PE (reuses CT): sc = w1^T @ C^T
        psc = p_ty.tile([1, 512], F32, tag="tiny", bufs=2)
        for k in range(KH):
            nc.tensor.matmul(psc[:], _r(w1c[k][:]),
                             _r(CT[k][:, ts(half, 512)]),
                             start=(k == 0), stop=(k == KH - 1))
        nc.scalar.copy(_r(scrow[:, ts(half, 512)]), psc[:])

        # S^T matmul (s_cq^T + s_c via rank-1) + fused bias/exp -> P'T
        pst = p_mm.tile([128, 512], F32, tag="mm", bufs=2)
        for k in range(KH):
            nc.tensor.matmul(
                pst[:], _r(QW3T[:, ts(k, 128)]), _r(CT[k][:, ts(half, 512)]),
                start=(k == 0), stop=False)
        nc.tensor.matmul(pst[:], _r(ones_row[:]), _r(scrow[:, ts(half, 512)]),
                         start=False, stop=True)
        # P'T = exp(s_cq^T + s_c + s_q + maskbias)
        nc.scalar.activation(_r(PT[:, ts(half, 512)]), pst[:], ACTF.Exp,
                             bias=sqe[:], scale=1.0)

        # E[i] = exp(maxS[i]) = rowmax of P' natural (PE transpose back)
        for tt in range(4):
            t = half * 4 + tt
            ppn = p_sm.tile([128, 128], F32, tag="sm", bufs=2)
            nc.tensor.transpose(_r(ppn[:]), _r(PT[:, ts(t, 128)]),
                                _r(ident[:]))
            nc.vector.reduce_max(_r(E[:, t:t + 1]), ppn[:], axis=AX.X)

        # Z' columns for softmax_j: batched ones-column matmuls
        pzg = p_ty.tile([128, 4], F32, tag="tiny", bufs=2)
        for tt in range(4):
            t = half * 4 + tt
            nc.tensor.matmul(pzg[:, tt:tt + 1], PT[:, ts(t, 128)], ones_col,
                             start=True, stop=True)
        nc.vector.reciprocal(RZP[:, ts(half, 4)], pzg[:])

        # A path per i-tile; stage [A|C*A], one 4KB-row DMA out
        for tt in range(4):
            t = half * 4 + tt
            pa = p_mm.tile([128, 512], F32, tag="mm", bufs=2)
            nc.tensor.matmul(pa[:], _r(PT[:, ts(t, 128)]), _r(Qsb[:]),
                             start=True, stop=True)
            ot = ot_pool.tile([128, 2 * H], F32, tag=f"ot{t}", bufs=2,
                              name=f"ot{t}_{b}")
            if t % 2 == 0:
                nc.scalar.mul(ot[:, 0:H], pa[:], RZP[:, t:t + 1])
            else:
                nc.vector.tensor_scalar_mul(ot[:, 0:H], pa[:],
                                            RZP[:, t:t + 1])
            if t % 2 == 0:
                nc.vector.tensor_mul(ot[:, H:2 * H], Csb[t][:], ot[:, 0:H])
            else:
                nc.gpsimd.tensor_mul(ot[:, H:2 * H], Csb[t][:], ot[:, 0:H])
            nc.sync.dma_start(O_ap[b, ts(t, 128), H:3 * H], ot[:])

    # ---- B path (needs full E): B_vec row via M=1 matmuls contracting i --
    pbrow = p_ty.tile([1, H], F32, tag="tiny", bufs=2)
    for t in range(NT):
        nc.tensor.matmul(pbrow[:], _r(E[:, t:t + 1]), _r(Csb[t][:]),
                         start=(t == 0), stop=(t == NT - 1))
    # Z2 = sum(E): free-dim reduce on DVE, partition reduce via one matmul
    rse = sm_pool.tile([128, 1], F32, tag="rse", bufs=2)
    nc.vector.reduce_sum(rse[:], E[:], axis=AX.X)
    pz2 = p_ty.tile([1, 1], F32, tag="tiny", bufs=2)
    nc.tensor.matmul(pz2[:], rse[:], ones_col, start=True, stop=True)
    rz2 = sm_pool.tile([1, 1], F32, tag="z2", bufs=2)
    nc.vector.reciprocal(rz2[:], pz2[:])
    # B_vec/Z2 row, then broadcast across partitions with a K=1 matmul
    Bts = sm_pool.tile([1, H], F32, tag="bts", bufs=2)
    nc.scalar.mul(_r(Bts[:]), pbrow[:], rz2[:])
    pbb = p_mm.tile([128, 512], F32, tag="mm", bufs=2)
    nc.tensor.matmul(pbb[:], _r(ones_row[:]), _r(Bts[:]), start=True,
                     stop=True)
    Bb = a_pool.tile([128, H], F32, tag="bb", bufs=2)
    nc.scalar.copy(Bb[:], pbb[:])

    # ---- C*B per i-tile: stage and DMA (2KB rows) ----
    for t in range(NT):
        cb = ot_pool.tile([128, H], F32, tag=f"cb{t % 4}", bufs=2,
                          name=f"cb{t}_{b}")
        if t % 2 == 0:
            nc.gpsimd.tensor_mul(cb[:], Csb[t][:], Bb[:])
        else:
            nc.vector.tensor_mul(cb[:], Csb[t][:], Bb[:])
        nc.sync.dma_start(O_ap[b, ts(t, 128), 3 * H:4 * H], cb[:])


def build_nc(n_rep: int = 1):
    nc = bacc.Bacc("TRN2", target_bir_lowering=False, debug=False,
                   num_devices=N_CORES)
    C_ap = nc.dram_tensor("C", [B_PER_CORE, CLEN, H], F32,
                          kind="ExternalInput").ap()
    Q_ap = nc.dram_tensor("Q", [B_PER_CORE, QLEN, H], F32,
                          kind="ExternalInput").ap()
    M_ap = nc.dram_tensor("q_mask", [B_PER_CORE, QLEN], I32,
                          kind="ExternalInput").ap()
    W_ap = nc.dram_tensor("w", [3 * H], F32, kind="ExternalInput").ap()
    ID_ap = nc.dram_tensor("ident", [128, 128], F32,
                           kind="ExternalInput").ap()
    O_ap = nc.dram_tensor("out", [B_PER_CORE, CLEN, 4 * H], F32,
                          kind="ExternalOutput").ap()

    with tile.TileContext(nc) as tc, ExitStack() as ctx:
        const_pool = ctx.enter_context(tc.tile_pool(name="const", bufs=1))
        c_pool = ctx.enter_context(tc.tile_pool(name="cpool", bufs=3))
        ct_pool = ctx.enter_context(tc.tile_pool(name="ctpool", bufs=2))
        q_pool = ctx.enter_context(tc.tile_pool(name="qpool", bufs=2))
        pt_pool = ctx.enter_context(tc.tile_pool(name="ptpool", bufs=2))
        sm_pool = ctx.enter_context(tc.tile_pool(name="smpool", bufs=2))
        scr_pool = ctx.enter_context(tc.tile_pool(name="scrpool", bufs=2))
        a_pool = ctx.enter_context(tc.tile_pool(name="apool", bufs=2))
        ot_pool = ctx.enter_context(tc.tile_pool(name="otpool", bufs=2))
        p_mm = ctx.enter_context(tc.tile_pool(name="pmm", bufs=2,
                                              space="PSUM"))
        p_ct = ctx.enter_context(tc.tile_pool(name="pct", bufs=2,
                                              space="PSUM"))
        p_sm = ctx.enter_context(tc.tile_pool(name="psm", bufs=2,
                                              space="PSUM"))
        p_ty = ctx.enter_context(tc.tile_pool(name="pty", bufs=2,
                                              space="PSUM"))

        # constants
        ident = const_pool.tile([128, 128], F32, tag="ident")
        nc.sync.dma_start(_r(ident[:]), ID_ap[:].bitcast(F32R))
        ones_raw = const_pool.tile([1, 128], F32, tag="ones_raw")
        nc.vector.memset(ones_raw[:], 1.0)
        ones_row = const_pool.tile([1, 128], F32, tag="ones_row")
        nc.vector.tensor_copy(_r(ones_row[:]), ones_raw[:])
        ones_col = nc.const_aps.tensor(1.0, (128, 1))
        wsb = const_pool.tile([128, 12], F32, tag="wsb")
        nc.sync.dma_start(_r(wsb[:]), W_ap.rearrange("(c p) -> p c", p=128).bitcast(F32R))
        w1c = [wsb[:, k:k + 1] for k in range(KH)]
        w3c = [wsb[:, 8 + k:9 + k] for k in range(KH)]
        w2r = const_pool.tile([1, H], F32, tag="w2r")
        nc.sync.dma_start(w2r[:], W_ap[H:2 * H].rearrange("(a h) -> a h", a=1))
        # broadcast w2 across partitions via K=1 matmul
        W2b = const_pool.tile([128, H], F32, tag="w2b")
        pw = p_mm.tile([128, 512], F32, tag="mm", bufs=2)
        nc.tensor.matmul(pw[:], ones_raw[:], w2r[:], start=True, stop=True)
        nc.vector.tensor_copy(W2b[:], pw[:])

        consts = (ident, ones_row, ones_col, w1c, w3c, W2b)
        pools = (c_pool, ct_pool, q_pool, pt_pool, sm_pool, scr_pool, a_pool,
                 ot_pool, p_mm, p_ct, p_sm, p_ty)
        aps = (C_ap, Q_ap, M_ap, O_ap, consts)

        for _rep in range(n_rep):
            for b in range(B_PER_CORE):
                _emit_example(nc, pools, aps, b)

    nc.compile()
    return nc


_NC_CACHE: dict = {}


def _get_nc(n_rep: int = 1):
    key = ("nc", n_rep)
    if key not in _NC_CACHE:
        _NC_CACHE[key] = build_nc(n_rep)
    return _NC_CACHE[key]


def make_in_maps(C, Q, q_mask, w):
    ident = np.eye(128, dtype=np.float32)
    w = np.ascontiguousarray(w, dtype=np.float32)
    in_maps = []
    for c in range(N_CORES):
        sl = slice(c * B_PER_CORE, (c + 1) * B_PER_CORE)
        in_maps.append({
            "C": np.ascontiguousarray(C[sl], dtype=np.float32),
            "Q": np.ascontiguousarray(Q[sl], dtype=np.float32),
            "q_mask": np.ascontiguousarray(q_mask[sl], dtype=np.int32),
            "w": w,
            "ident": ident,
        })
    return in_maps


def kernel(C, Q, q_mask, w):
    nc = _get_nc(1)
    in_maps = make_in_maps(C, Q, q_mask, w)
    res = run_bass_kernel_spmd(nc, in_maps, list(range(N_CORES)))
    out = np.concatenate([res.results[c]["out"] for c in range(N_CORES)],
                         axis=0)
    return out
